# revision 34
# baseline (speedup 1.0000x reference)
"""Trainium2 Bass kernel for ContinuousCWTLayer (B=4, C=16, T=2048, F=32, TOK=256).

Strategy (8 NeuronCores, uniform SPMD program):
  - core i handles batch b=i//2, token-half i%2 (128 tokens), as 4 "units" x 32 tokens.
  - Depthwise CWT conv as im2col matmuls: contraction over the wavelet k-axis
    (2048 taps on partitions, 16 tiles of 128), M=128 weight columns =
    64 wavelet columns (32 freqs x {cos,sin}) x 2 time-shifts {0,1}; the two
    shifts produce conv at both bilinear taps (x0, x0+1) from ONE rhs stream.
  - The L1-normalized Morlet bank is precomputed on HOST in float64.
  - All input DMAs issue from the Sync queue (descriptor-gen off the ACT
    engine); per-unit combine is 6 DVE ops writing row slices of batched
    R/I tiles; the mag/phase tail runs ONCE per row-half (rows 0:64 after
    unit 1 -- overlapped with units 2-3 matmuls -- rows 64:128 at the end)
    with only 2 ACT table loads (Sqrt block, then Arctan).
"""

import math

import numpy as np

import concourse.bass as bass
import concourse.mybir as mybir
from concourse.bass_utils import run_bass_kernel_spmd
from concourse.tile import TileContext

B, C, MAX_T, F, TOK = 4, 16, 2048, 32, 256
QT = 16           # k tiles (contraction 2048 = 16 x 128)
U = 4             # units per core
JPU = 32          # tokens per unit
NCOL = JPU * C    # 512 matmul N columns per unit
XROWS = 4096      # padded, transposed x rows

f32 = mybir.dt.float32
f16 = mybir.dt.float16
f32r = mybir.dt.float32r
AF = mybir.ActivationFunctionType
ALU = mybir.AluOpType

SQRT2 = float(np.float32(math.sqrt(2.0)))
FOUR_OVER_PI = float(np.float32(4.0 / math.pi))
TWO_OVER_PI = float(np.float32(2.0 / math.pi))

_NC_CACHE = {}


def _split_multiwaits(nc, wlimit=1, ulimit=99):
    """Hoist excess attached sem-waits/updates onto separate same-engine
    InstNoOp instructions.  The walrus build in this container encodes at
    most one sync-wait command per instruction; Tile attaches several."""
    n_new = 0
    for f in nc.m.functions:
        for bb in f.blocks:
            new = []
            for inst in bb.instructions:
                si = inst.sync_info
                if si is not None and si.on_wait and len(si.on_wait) > wlimit:
                    waits = list(si.on_wait)
                    extra, keep = waits[:-wlimit], waits[-wlimit:]
                    for i in range(0, len(extra), wlimit):
                        nop = mybir.InstNoOp(
                            name=nc.get_next_instruction_name(),
                            engine=inst.engine,
                            bass_nofuse=True,
                            sync_info=mybir.SyncInfo(
                                on_wait=extra[i:i + wlimit], on_update=[]),
                        )
                        new.append(nop)
                        n_new += 1
                    inst.sync_info = mybir.SyncInfo(
                        on_wait=keep, on_update=list(si.on_update or []))
                new.append(inst)
                si = inst.sync_info
                if si is not None and si.on_update and len(si.on_update) > ulimit:
                    ups = list(si.on_update)
                    keep, extra = ups[:ulimit], ups[ulimit:]
                    inst.sync_info = mybir.SyncInfo(
                        on_wait=list(si.on_wait or []), on_update=keep)
                    for i in range(0, len(extra), ulimit):
                        nop = mybir.InstNoOp(
                            name=nc.get_next_instruction_name(),
                            engine=inst.engine,
                            bass_nofuse=True,
                            sync_info=mybir.SyncInfo(
                                on_wait=[], on_update=extra[i:i + ulimit]),
                        )
                        new.append(nop)
                        n_new += 1
            bb.instructions = new
    return n_new


def _build_nc(split=True):
    nc = bass.Bass()
    xim = nc.declare_dram_parameter("xim", [U, 128, QT * NCOL], f32, isOutput=False)
    wbank = nc.declare_dram_parameter("wbank", [128, QT * 128], f32, isOutput=False)
    wx = nc.declare_dram_parameter("wx", [F, 2 * U * NCOL], f32, isOutput=False)
    out = nc.declare_dram_parameter("out", [2, 128, NCOL], f32, isOutput=True)

    UN = U * NCOL

    with TileContext(nc) as tc:
        with (
            tc.tile_pool(name="const", bufs=1) as cpool,
            tc.tile_pool(name="rpool", bufs=2) as rp,
            tc.tile_pool(name="comb", bufs=2) as cb,
            tc.tile_pool(name="tail", bufs=1) as tp,
            tc.tile_pool(name="psum", bufs=2, space="PSUM") as pp,
        ):
            # host-precomputed normalized wavelet bank
            wb = cpool.tile([128, QT * 128], f32)
            nc.sync.dma_start(out=wb[:], in_=wbank[:, :])

            # bilinear combine weights (DMAs issued inside the unit-0 chunk
            # chain below so the initial transfer burst stays small)
            wxt0 = cpool.tile([F, UN], f32)
            wxt1 = cpool.tile([F, UN], f32)

            eps = cpool.tile([128, 1], f32)
            nc.vector.memset(eps[:], 1e-8)


            # batched real/imag accumulators: row u*32+f
            R128 = cpool.tile([128, NCOL], f32)
            I128 = cpool.tile([128, NCOL], f32)

            # tail tiles, written per row-half
            mg = tp.tile([128, NCOL], f32, tag="mg")
            m0 = tp.tile([128, NCOL], f32, tag="m0")
            sqr = tp.tile([128, NCOL], f32, tag="sqr")
            sqi = tp.tile([128, NCOL], f32, tag="sqi")
            ss = tp.tile([128, NCOL], f32, tag="ss")
            dmr = tp.tile([128, NCOL], f32, tag="dmr")
            dpr = tp.tile([128, NCOL], f32, tag="dpr")
            n1 = tp.tile([128, NCOL], f32, tag="n1")
            d1 = tp.tile([128, NCOL], f32, tag="d1")
            d2 = tp.tile([128, NCOL], f32, tag="d2")
            den = tp.tile([128, NCOL], f32, tag="den")
            inv = tp.tile([128, NCOL], f32, tag="inv")
            qq = tp.tile([128, NCOL], f32, tag="qq")
            at = tp.tile([128, NCOL], f32, tag="at")
            sg = tp.tile([128, NCOL], f32, tag="sg")
            ph = tp.tile([128, NCOL], f32, tag="ph")

            def tail_half(p):
                """mag/phase for rows hs:hs+64 (units 2p, 2p+1). All SBUF
                operand bases equal hs (walrus base-partition rule)."""
                hs = slice(64 * p, 64 * (p + 1))
                R = R128[hs, :]
                I = I128[hs, :]
                nc.vector.tensor_tensor(out=sqr[hs, :], in0=R, in1=R,
                                        op=ALU.mult)
                nc.vector.tensor_tensor(out=sqi[hs, :], in0=I, in1=I,
                                        op=ALU.mult)
                # sign(i) as {-1,+1} via compares (off the critical path)
                nc.vector.tensor_scalar(out=sg[hs, :], in0=I, scalar1=0.0,
                                        scalar2=2.0, op0=ALU.is_ge,
                                        op1=ALU.mult)
                nc.vector.tensor_scalar(out=sg[hs, :], in0=sg[hs, :],
                                        scalar1=-1.0, scalar2=None,
                                        op0=ALU.add)
                nc.vector.tensor_tensor(out=ss[hs, :], in0=sqr[hs, :],
                                        in1=sqi[hs, :], op=ALU.add)
                nc.scalar.activation(mg[hs, :], ss[hs, :], AF.Sqrt,
                                     bias=eps[hs, :])
                # eps-free magnitude for the quarter-angle phase path (the
                # biased mag skews qq when ss ~ eps)
                nc.scalar.activation(m0[hs, :], ss[hs, :], AF.Sqrt)
                nc.vector.tensor_tensor(out=dmr[hs, :], in0=m0[hs, :], in1=R,
                                        op=ALU.subtract)
                nc.vector.tensor_scalar(out=dmr[hs, :], in0=dmr[hs, :],
                                        scalar1=0.0, scalar2=None, op0=ALU.max)
                nc.vector.tensor_tensor(out=dpr[hs, :], in0=m0[hs, :], in1=R,
                                        op=ALU.add)
                nc.vector.tensor_scalar(out=dpr[hs, :], in0=dpr[hs, :],
                                        scalar1=0.0, scalar2=None, op0=ALU.max)
                nc.scalar.activation(n1[hs, :], dmr[hs, :], AF.Sqrt)
                nc.scalar.activation(d1[hs, :], dpr[hs, :], AF.Sqrt)
                nc.scalar.activation(d2[hs, :], m0[hs, :], AF.Sqrt, scale=2.0)
                nc.sync.dma_start(out=out[0, 64 * p:64 * (p + 1), :],
                                  in_=mg[hs, :])
                # the den->atan->phase segment: full-width for the hidden
                # first half; two pipelined column halves (function-major,
                # one Arctan table load) for the exposed second half
                H = NCOL // 2
                chunks = [slice(0, NCOL)] if p == 0 else [slice(0, H),
                                                          slice(H, NCOL)]
                for cs in chunks:
                    nc.vector.tensor_tensor(out=den[hs, cs], in0=d1[hs, cs],
                                            in1=d2[hs, cs], op=ALU.add)
                    hw = (cs.start + cs.stop) // 2
                    nc.vector.reciprocal(inv[hs, cs.start:hw],
                                         den[hs, cs.start:hw])
                    nc.vector.reciprocal(inv[hs, hw:cs.stop],
                                         den[hs, hw:cs.stop])
                    nc.vector.tensor_tensor(out=qq[hs, cs], in0=n1[hs, cs],
                                            in1=inv[hs, cs], op=ALU.mult)
                for cs in chunks:
                    nc.scalar.activation(at[hs, cs], qq[hs, cs], AF.Arctan)
                    nc.vector.scalar_tensor_tensor(
                        out=ph[hs, cs], in0=at[hs, cs], scalar=FOUR_OVER_PI,
                        in1=sg[hs, cs], op0=ALU.mult, op1=ALU.mult)
                    nc.sync.dma_start(out=out[1, 64 * p:64 * (p + 1), cs],
                                      in_=ph[hs, cs])
                if p == 0:
                    # dummy op: preload the Sqrt table for the second half
                    # while the matmuls still run (off the critical path)
                    nc.scalar.activation(d2[hs.start:hs.start + 1, 0:1],
                                         eps[hs.start:hs.start + 1, :],
                                         AF.Sqrt)

            for u in range(U):
                # per-unit im2col in 4 chunk tiles of 4 q-tiles (1.05MB
                # each; separate tiles => matmuls start as chunks land).
                # ALL DMAs issue from the Sync engine: same-engine HWDGE
                # transfers complete in FIFO order, so the first chunk lands
                # first (splitting across queues round-robins the rings and
                # delays the critical first chunk).
                qc = 4 * NCOL
                Rg = []
                for g in range(4):
                    Rt = rp.tile([128, qc], f32, tag=f"Rg{g}")
                    nc.sync.dma_start(out=Rt[:], in_=xim[u, :, g * qc:(g + 1) * qc])
                    Rg.append(Rt)
                if u == 0:
                    nc.sync.dma_start(out=wxt0[:], in_=wx[:, 0:UN])
                    nc.sync.dma_start(out=wxt1[:], in_=wx[:, UN:2 * UN])

                ps = pp.tile([128, NCOL], f32, tag="ps")
                for q in range(QT):
                    src = Rg[q // 4]
                    nc.tensor.matmul(
                        ps[:],
                        lhsT=wb[:, q * 128:(q + 1) * 128],
                        rhs=src[:, (q % 4) * NCOL:(q % 4 + 1) * NCOL],
                        start=(q == 0), stop=(q == QT - 1),
                    )

                # bilinear combine into R128/I128 row slices (psum operands
                # are exempt from the SBUF base-partition-match rule)
                ucols = slice(u * NCOL, (u + 1) * NCOL)
                lor = cb.tile([F, NCOL], f32, tag="lor")
                nc.vector.tensor_tensor(out=lor[:], in0=ps[0:32, :],
                                        in1=wxt0[:, ucols], op=ALU.mult)
                loi = cb.tile([F, NCOL], f32, tag="loi")
                nc.vector.tensor_tensor(out=loi[:], in0=ps[32:64, :],
                                        in1=wxt0[:, ucols], op=ALU.mult)
                hir = cb.tile([F, NCOL], f32, tag="hir")
                nc.vector.tensor_tensor(out=hir[:], in0=ps[64:96, :],
                                        in1=wxt1[:, ucols], op=ALU.mult)
                hii = cb.tile([F, NCOL], f32, tag="hii")
                nc.vector.tensor_tensor(out=hii[:], in0=ps[96:128, :],
                                        in1=wxt1[:, ucols], op=ALU.mult)
                nc.vector.tensor_tensor(out=R128[u * F:(u + 1) * F, :],
                                        in0=lor[:], in1=hir[:], op=ALU.add)
                nc.vector.tensor_tensor(out=I128[u * F:(u + 1) * F, :],
                                        in0=loi[:], in1=hii[:], op=ALU.add)

                if u == 1:
                    tail_half(0)   # rows 0:64, overlaps units 2-3 matmuls
            tail_half(1)           # rows 64:128
    if split:
        _split_multiwaits(nc, wlimit=1)
    return nc


def _get_nc(split=True):
    key = ("nc", split)
    if key not in _NC_CACHE:
        _NC_CACHE[key] = _build_nc(split=split)
    return _NC_CACHE[key]


def _host_wbank(fsb, freqs, n_cycles):
    """Normalized Morlet bank [128, QT*128] in f64; col m = s*64 + ri*32 + f,
    tap index i = 128*q + dk, wavelet argument t_rel = i - s - 1024."""
    f = np.maximum(freqs.astype(np.float64), 0.1)
    ncv = np.maximum(n_cycles.astype(np.float64), 1.0)
    sigma = ncv / (2.0 * math.pi * f)
    i = np.arange(2048, dtype=np.float64)
    wb = np.empty((2048, 128), np.float64)
    for s in range(2):
        t_sec = (i[:, None] - s - 1024.0) / fsb            # (2048, F)
        env = np.exp(-t_sec ** 2 / (2.0 * sigma[None, :] ** 2))
        norm = env.sum(0) + 1e-8
        wb[:, s * 64:s * 64 + 32] = np.cos(2.0 * math.pi * f[None, :] * t_sec) * env / norm
        wb[:, s * 64 + 32:s * 64 + 64] = np.sin(2.0 * math.pi * f[None, :] * t_sec) * env / norm
    # [i, m] -> [dk, q*128 + m]
    return np.ascontiguousarray(
        wb.reshape(QT, 128, 128).transpose(1, 0, 2).reshape(128, QT * 128)
    ).astype(np.float32)


def _host_prep(x, fs, seq_lens, freqs, n_cycles):
    """Per-core input maps. Pure layout + O(F*K) host wavelet-bank prep."""
    x = np.asarray(x, np.float32)
    fs = np.asarray(fs, np.float32)
    seq_lens = np.asarray(seq_lens)
    freqs = np.asarray(freqs, np.float32)
    n_cycles = np.asarray(n_cycles, np.float32)

    f1 = np.float32(1.0)
    # token sample positions, bit-exact with the reference's f32 math
    steps = np.arange(TOK, dtype=np.float32) * np.float32(1.0 / (TOK - 1))
    in_maps = []
    per_core_meta = []
    wbank_cache = {}
    for core in range(8):
        b = core // 2
        half = core % 2
        L = np.float32(seq_lens[b])
        end_x = np.float32(-1.0) + np.float32(2.0) * (L - f1) / np.float32(MAX_T - 1)
        x_coords = np.float32(-1.0) + steps * (end_x + f1)
        px = (x_coords + f1) * np.float32(0.5) * np.float32(MAX_T - 1)
        x0f = np.floor(px)
        wx1 = px - x0f
        wx0 = f1 - wx1
        x0 = x0f.astype(np.int64)
        oob = (x0 + 1) > (MAX_T - 1)
        wx1 = np.where(oob, np.float32(0.0), wx1)

        toks = np.arange(half * 128, half * 128 + 128)
        x0c = x0[toks]
        wx0c = wx0[toks].astype(np.float32)
        wx1c = wx1[toks].astype(np.float32)

        # padded transposed x: rows [1024, 3072) hold x[b].T
        xpad = np.zeros((XROWS, C), np.float32)
        xpad[1024:1024 + MAX_T, :] = x[b].T.astype(np.float32)

        # im2col, q-major: xim[u, dk, q*512 + j*16 + c] = xpad[x0 + 128 q + dk, c]
        xim = np.empty((U, 128, QT, JPU, C), np.float32)
        for uu in range(U):
            for jj in range(JPU):
                w = xpad[x0c[uu * JPU + jj]: x0c[uu * JPU + jj] + 2048, :]
                xim[uu, :, :, jj, :] = w.reshape(QT, 128, C).transpose(1, 0, 2)
        xim = np.ascontiguousarray(xim.reshape(U, 128, QT * JPU * C))

        if b not in wbank_cache:
            wbank_cache[b] = _host_wbank(float(fs[b]), freqs, n_cycles)
        wbank = wbank_cache[b]

        # combine weights; col = u*512 + j*16 + c; first U*NCOL cols = wx0
        # (shift-0), next U*NCOL = wx1 (shift-1); replicated over 32 rows
        wxa = np.empty((F, 2 * U * NCOL), np.float32)
        for uu in range(U):
            w0 = np.repeat(wx0c[uu * JPU:(uu + 1) * JPU], C)
            w1 = np.repeat(wx1c[uu * JPU:(uu + 1) * JPU], C)
            wxa[:, uu * NCOL:(uu + 1) * NCOL] = np.broadcast_to(w0, (F, NCOL))
            wxa[:, U * NCOL + uu * NCOL:U * NCOL + (uu + 1) * NCOL] = \
                np.broadcast_to(w1, (F, NCOL))
        in_maps.append({"xim": xim, "wbank": wbank, "wx": wxa})
        per_core_meta.append((b, half))
    return in_maps, per_core_meta


def _assemble(results, per_core_meta):
    full = np.empty((B, C, 2, F, TOK), np.float32)
    for core, (b, half) in enumerate(per_core_meta):
        # out[ch, u*32+f, j*16+c] -> full[b, c, ch, f, half*128 + u*32 + j]
        o = np.asarray(results[core]["out"]).reshape(2, U, F, JPU, C)
        o2 = o.transpose(4, 0, 2, 1, 3).reshape(C, 2, F, 128)
        full[b, :, :, :, half * 128:(half + 1) * 128] = o2
    return full


def kernel(x, fs, seq_lens, freqs, n_cycles, target_time_tokens):
    assert int(target_time_tokens) == TOK
    nc = _get_nc()
    in_maps, meta = _host_prep(x, fs, seq_lens, freqs, n_cycles)
    res = run_bass_kernel_spmd(nc, in_maps, list(range(8)))
    return _assemble(res.results, meta)


# revision 35
# speedup vs baseline: 1.0068x; 1.0068x over previous
"""Trainium2 Bass kernel for ContinuousCWTLayer (B=4, C=16, T=2048, F=32, TOK=256).

Strategy (8 NeuronCores, uniform SPMD program):
  - core i handles batch b=i//2, token-half i%2 (128 tokens), as 4 "units" x 32 tokens.
  - Depthwise CWT conv as im2col matmuls: contraction over the wavelet k-axis
    (2048 taps on partitions, 16 tiles of 128), M=128 weight columns =
    64 wavelet columns (32 freqs x {cos,sin}) x 2 time-shifts {0,1}; the two
    shifts produce conv at both bilinear taps (x0, x0+1) from ONE rhs stream.
  - The L1-normalized Morlet bank is precomputed on HOST in float64.
  - All input DMAs issue from the Sync queue (descriptor-gen off the ACT
    engine); per-unit combine is 6 DVE ops writing row slices of batched
    R/I tiles; the mag/phase tail runs ONCE per row-half (rows 0:64 after
    unit 1 -- overlapped with units 2-3 matmuls -- rows 64:128 at the end)
    with only 2 ACT table loads (Sqrt block, then Arctan).
"""

import math

import numpy as np

import concourse.bass as bass
import concourse.mybir as mybir
from concourse.bass_utils import run_bass_kernel_spmd
from concourse.tile import TileContext

B, C, MAX_T, F, TOK = 4, 16, 2048, 32, 256
QT = 16           # k tiles (contraction 2048 = 16 x 128)
U = 4             # units per core
JPU = 32          # tokens per unit
NCOL = JPU * C    # 512 matmul N columns per unit
XROWS = 4096      # padded, transposed x rows

f32 = mybir.dt.float32
f16 = mybir.dt.float16
f32r = mybir.dt.float32r
AF = mybir.ActivationFunctionType
ALU = mybir.AluOpType

SQRT2 = float(np.float32(math.sqrt(2.0)))
FOUR_OVER_PI = float(np.float32(4.0 / math.pi))
TWO_OVER_PI = float(np.float32(2.0 / math.pi))

_NC_CACHE = {}


def _split_multiwaits(nc, wlimit=1, ulimit=99):
    """Hoist excess attached sem-waits/updates onto separate same-engine
    InstNoOp instructions.  The walrus build in this container encodes at
    most one sync-wait command per instruction; Tile attaches several."""
    n_new = 0
    for f in nc.m.functions:
        for bb in f.blocks:
            new = []
            for inst in bb.instructions:
                si = inst.sync_info
                if si is not None and si.on_wait and len(si.on_wait) > wlimit:
                    waits = list(si.on_wait)
                    extra, keep = waits[:-wlimit], waits[-wlimit:]
                    for i in range(0, len(extra), wlimit):
                        nop = mybir.InstNoOp(
                            name=nc.get_next_instruction_name(),
                            engine=inst.engine,
                            bass_nofuse=True,
                            sync_info=mybir.SyncInfo(
                                on_wait=extra[i:i + wlimit], on_update=[]),
                        )
                        new.append(nop)
                        n_new += 1
                    inst.sync_info = mybir.SyncInfo(
                        on_wait=keep, on_update=list(si.on_update or []))
                new.append(inst)
                si = inst.sync_info
                if si is not None and si.on_update and len(si.on_update) > ulimit:
                    ups = list(si.on_update)
                    keep, extra = ups[:ulimit], ups[ulimit:]
                    inst.sync_info = mybir.SyncInfo(
                        on_wait=list(si.on_wait or []), on_update=keep)
                    for i in range(0, len(extra), ulimit):
                        nop = mybir.InstNoOp(
                            name=nc.get_next_instruction_name(),
                            engine=inst.engine,
                            bass_nofuse=True,
                            sync_info=mybir.SyncInfo(
                                on_wait=[], on_update=extra[i:i + ulimit]),
                        )
                        new.append(nop)
                        n_new += 1
            bb.instructions = new
    return n_new


def _build_nc(split=True):
    nc = bass.Bass()
    xim = nc.declare_dram_parameter("xim", [U, 128, QT * NCOL], f32, isOutput=False)
    wbank = nc.declare_dram_parameter("wbank", [128, QT * 128], f32, isOutput=False)
    wx = nc.declare_dram_parameter("wx", [F, 2 * U * NCOL], f32, isOutput=False)
    out = nc.declare_dram_parameter("out", [2, 128, NCOL], f32, isOutput=True)

    UN = U * NCOL

    with TileContext(nc) as tc:
        with (
            tc.tile_pool(name="const", bufs=1) as cpool,
            tc.tile_pool(name="rpool", bufs=2) as rp,
            tc.tile_pool(name="comb", bufs=2) as cb,
            tc.tile_pool(name="tail", bufs=1) as tp,
            tc.tile_pool(name="psum", bufs=2, space="PSUM") as pp,
        ):
            # host-precomputed normalized wavelet bank
            wb = cpool.tile([128, QT * 128], f32)
            nc.sync.dma_start(out=wb[:], in_=wbank[:, :])

            # bilinear combine weights (DMAs issued inside the unit-0 chunk
            # chain below so the initial transfer burst stays small)
            wxt0 = cpool.tile([F, UN], f32)
            wxt1 = cpool.tile([F, UN], f32)

            eps = cpool.tile([128, 1], f32)
            nc.vector.memset(eps[:], 1e-8)


            # batched real/imag accumulators: row u*32+f
            R128 = cpool.tile([128, NCOL], f32)
            I128 = cpool.tile([128, NCOL], f32)

            # tail tiles, written per row-half
            mg = tp.tile([128, NCOL], f32, tag="mg")
            m0 = tp.tile([128, NCOL], f32, tag="m0")
            sqr = tp.tile([128, NCOL], f32, tag="sqr")
            sqi = tp.tile([128, NCOL], f32, tag="sqi")
            ss = tp.tile([128, NCOL], f32, tag="ss")
            dmr = tp.tile([128, NCOL], f32, tag="dmr")
            dpr = tp.tile([128, NCOL], f32, tag="dpr")
            n1 = tp.tile([128, NCOL], f32, tag="n1")
            d1 = tp.tile([128, NCOL], f32, tag="d1")
            d2 = tp.tile([128, NCOL], f32, tag="d2")
            den = tp.tile([128, NCOL], f32, tag="den")
            inv = tp.tile([128, NCOL], f32, tag="inv")
            qq = tp.tile([128, NCOL], f32, tag="qq")
            at = tp.tile([128, NCOL], f32, tag="at")
            sg = tp.tile([128, NCOL], f32, tag="sg")
            ph = tp.tile([128, NCOL], f32, tag="ph")

            def tail_half(p):
                """mag/phase for rows hs:hs+64 (units 2p, 2p+1). All SBUF
                operand bases equal hs (walrus base-partition rule)."""
                hs = slice(64 * p, 64 * (p + 1))
                R = R128[hs, :]
                I = I128[hs, :]
                nc.vector.tensor_tensor(out=sqr[hs, :], in0=R, in1=R,
                                        op=ALU.mult)
                nc.vector.tensor_tensor(out=sqi[hs, :], in0=I, in1=I,
                                        op=ALU.mult)
                # sign(i) as {-1,+1} via compares (off the critical path)
                nc.vector.tensor_scalar(out=sg[hs, :], in0=I, scalar1=0.0,
                                        scalar2=2.0, op0=ALU.is_ge,
                                        op1=ALU.mult)
                nc.vector.tensor_scalar(out=sg[hs, :], in0=sg[hs, :],
                                        scalar1=-1.0, scalar2=None,
                                        op0=ALU.add)
                nc.vector.tensor_tensor(out=ss[hs, :], in0=sqr[hs, :],
                                        in1=sqi[hs, :], op=ALU.add)
                nc.scalar.activation(mg[hs, :], ss[hs, :], AF.Sqrt,
                                     bias=eps[hs, :])
                # eps-free magnitude for the quarter-angle phase path (the
                # biased mag skews qq when ss ~ eps)
                nc.scalar.activation(m0[hs, :], ss[hs, :], AF.Sqrt)
                nc.vector.tensor_tensor(out=dmr[hs, :], in0=m0[hs, :], in1=R,
                                        op=ALU.subtract)
                nc.vector.tensor_scalar(out=dmr[hs, :], in0=dmr[hs, :],
                                        scalar1=0.0, scalar2=None, op0=ALU.max)
                nc.vector.tensor_tensor(out=dpr[hs, :], in0=m0[hs, :], in1=R,
                                        op=ALU.add)
                nc.vector.tensor_scalar(out=dpr[hs, :], in0=dpr[hs, :],
                                        scalar1=0.0, scalar2=None, op0=ALU.max)
                nc.scalar.activation(n1[hs, :], dmr[hs, :], AF.Sqrt)
                nc.scalar.activation(d1[hs, :], dpr[hs, :], AF.Sqrt)
                nc.scalar.activation(d2[hs, :], m0[hs, :], AF.Sqrt, scale=2.0)
                nc.sync.dma_start(out=out[0, 64 * p:64 * (p + 1), :],
                                  in_=mg[hs, :])
                nc.vector.tensor_tensor(out=den[hs, :], in0=d1[hs, :],
                                        in1=d2[hs, :], op=ALU.add)
                # reciprocal split by column halves (cost scales with free
                # size) so qq/at start on the first half sooner
                nc.vector.reciprocal(inv[hs, 0:NCOL // 2], den[hs, 0:NCOL // 2])
                nc.vector.reciprocal(inv[hs, NCOL // 2:], den[hs, NCOL // 2:])
                nc.vector.tensor_tensor(out=qq[hs, :], in0=n1[hs, :],
                                        in1=inv[hs, :], op=ALU.mult)
                nc.scalar.activation(at[hs, :], qq[hs, :], AF.Arctan)
                if p == 0:
                    # dummy op: preload the Sqrt table for the second half
                    # while the matmuls still run (off the critical path)
                    nc.scalar.activation(d2[hs.start:hs.start + 1, 0:1],
                                         eps[hs.start:hs.start + 1, :],
                                         AF.Sqrt)
                nc.vector.scalar_tensor_tensor(
                    out=ph[hs, :], in0=at[hs, :], scalar=FOUR_OVER_PI,
                    in1=sg[hs, :], op0=ALU.mult, op1=ALU.mult)
                nc.sync.dma_start(out=out[1, 64 * p:64 * (p + 1), :],
                                  in_=ph[hs, :])

            for u in range(U):
                # per-unit im2col in 4 chunk tiles of 4 q-tiles (1.05MB
                # each; separate tiles => matmuls start as chunks land).
                # ALL DMAs issue from the Sync engine: same-engine HWDGE
                # transfers complete in FIFO order, so the first chunk lands
                # first (splitting across queues round-robins the rings and
                # delays the critical first chunk).
                qc = 4 * NCOL
                Rg = []
                for g in range(4):
                    Rt = rp.tile([128, qc], f32, tag=f"Rg{g}")
                    nc.sync.dma_start(out=Rt[:], in_=xim[u, :, g * qc:(g + 1) * qc])
                    Rg.append(Rt)
                if u == 0:
                    nc.sync.dma_start(out=wxt0[:], in_=wx[:, 0:UN])
                    nc.sync.dma_start(out=wxt1[:], in_=wx[:, UN:2 * UN])

                ps = pp.tile([128, NCOL], f32, tag="ps")
                for q in range(QT):
                    src = Rg[q // 4]
                    nc.tensor.matmul(
                        ps[:],
                        lhsT=wb[:, q * 128:(q + 1) * 128],
                        rhs=src[:, (q % 4) * NCOL:(q % 4 + 1) * NCOL],
                        start=(q == 0), stop=(q == QT - 1),
                    )

                # bilinear combine into R128/I128 row slices (psum operands
                # are exempt from the SBUF base-partition-match rule)
                ucols = slice(u * NCOL, (u + 1) * NCOL)
                lor = cb.tile([F, NCOL], f32, tag="lor")
                nc.vector.tensor_tensor(out=lor[:], in0=ps[0:32, :],
                                        in1=wxt0[:, ucols], op=ALU.mult)
                loi = cb.tile([F, NCOL], f32, tag="loi")
                nc.vector.tensor_tensor(out=loi[:], in0=ps[32:64, :],
                                        in1=wxt0[:, ucols], op=ALU.mult)
                hir = cb.tile([F, NCOL], f32, tag="hir")
                nc.vector.tensor_tensor(out=hir[:], in0=ps[64:96, :],
                                        in1=wxt1[:, ucols], op=ALU.mult)
                hii = cb.tile([F, NCOL], f32, tag="hii")
                nc.vector.tensor_tensor(out=hii[:], in0=ps[96:128, :],
                                        in1=wxt1[:, ucols], op=ALU.mult)
                nc.vector.tensor_tensor(out=R128[u * F:(u + 1) * F, :],
                                        in0=lor[:], in1=hir[:], op=ALU.add)
                nc.vector.tensor_tensor(out=I128[u * F:(u + 1) * F, :],
                                        in0=loi[:], in1=hii[:], op=ALU.add)

                if u == 1:
                    tail_half(0)   # rows 0:64, overlaps units 2-3 matmuls
            tail_half(1)           # rows 64:128
    if split:
        _split_multiwaits(nc, wlimit=1)
    return nc


def _get_nc(split=True):
    key = ("nc", split)
    if key not in _NC_CACHE:
        _NC_CACHE[key] = _build_nc(split=split)
    return _NC_CACHE[key]


def _host_wbank(fsb, freqs, n_cycles):
    """Normalized Morlet bank [128, QT*128] in f64; col m = s*64 + ri*32 + f,
    tap index i = 128*q + dk, wavelet argument t_rel = i - s - 1024."""
    f = np.maximum(freqs.astype(np.float64), 0.1)
    ncv = np.maximum(n_cycles.astype(np.float64), 1.0)
    sigma = ncv / (2.0 * math.pi * f)
    i = np.arange(2048, dtype=np.float64)
    wb = np.empty((2048, 128), np.float64)
    for s in range(2):
        t_sec = (i[:, None] - s - 1024.0) / fsb            # (2048, F)
        env = np.exp(-t_sec ** 2 / (2.0 * sigma[None, :] ** 2))
        norm = env.sum(0) + 1e-8
        wb[:, s * 64:s * 64 + 32] = np.cos(2.0 * math.pi * f[None, :] * t_sec) * env / norm
        wb[:, s * 64 + 32:s * 64 + 64] = np.sin(2.0 * math.pi * f[None, :] * t_sec) * env / norm
    # [i, m] -> [dk, q*128 + m]
    return np.ascontiguousarray(
        wb.reshape(QT, 128, 128).transpose(1, 0, 2).reshape(128, QT * 128)
    ).astype(np.float32)


def _host_prep(x, fs, seq_lens, freqs, n_cycles):
    """Per-core input maps. Pure layout + O(F*K) host wavelet-bank prep."""
    x = np.asarray(x, np.float32)
    fs = np.asarray(fs, np.float32)
    seq_lens = np.asarray(seq_lens)
    freqs = np.asarray(freqs, np.float32)
    n_cycles = np.asarray(n_cycles, np.float32)

    f1 = np.float32(1.0)
    # token sample positions, bit-exact with the reference's f32 math
    steps = np.arange(TOK, dtype=np.float32) * np.float32(1.0 / (TOK - 1))
    in_maps = []
    per_core_meta = []
    wbank_cache = {}
    for core in range(8):
        b = core // 2
        half = core % 2
        L = np.float32(seq_lens[b])
        end_x = np.float32(-1.0) + np.float32(2.0) * (L - f1) / np.float32(MAX_T - 1)
        x_coords = np.float32(-1.0) + steps * (end_x + f1)
        px = (x_coords + f1) * np.float32(0.5) * np.float32(MAX_T - 1)
        x0f = np.floor(px)
        wx1 = px - x0f
        wx0 = f1 - wx1
        x0 = x0f.astype(np.int64)
        oob = (x0 + 1) > (MAX_T - 1)
        wx1 = np.where(oob, np.float32(0.0), wx1)

        toks = np.arange(half * 128, half * 128 + 128)
        x0c = x0[toks]
        wx0c = wx0[toks].astype(np.float32)
        wx1c = wx1[toks].astype(np.float32)

        # padded transposed x: rows [1024, 3072) hold x[b].T
        xpad = np.zeros((XROWS, C), np.float32)
        xpad[1024:1024 + MAX_T, :] = x[b].T.astype(np.float32)

        # im2col, q-major: xim[u, dk, q*512 + j*16 + c] = xpad[x0 + 128 q + dk, c]
        xim = np.empty((U, 128, QT, JPU, C), np.float32)
        for uu in range(U):
            for jj in range(JPU):
                w = xpad[x0c[uu * JPU + jj]: x0c[uu * JPU + jj] + 2048, :]
                xim[uu, :, :, jj, :] = w.reshape(QT, 128, C).transpose(1, 0, 2)
        xim = np.ascontiguousarray(xim.reshape(U, 128, QT * JPU * C))

        if b not in wbank_cache:
            wbank_cache[b] = _host_wbank(float(fs[b]), freqs, n_cycles)
        wbank = wbank_cache[b]

        # combine weights; col = u*512 + j*16 + c; first U*NCOL cols = wx0
        # (shift-0), next U*NCOL = wx1 (shift-1); replicated over 32 rows
        wxa = np.empty((F, 2 * U * NCOL), np.float32)
        for uu in range(U):
            w0 = np.repeat(wx0c[uu * JPU:(uu + 1) * JPU], C)
            w1 = np.repeat(wx1c[uu * JPU:(uu + 1) * JPU], C)
            wxa[:, uu * NCOL:(uu + 1) * NCOL] = np.broadcast_to(w0, (F, NCOL))
            wxa[:, U * NCOL + uu * NCOL:U * NCOL + (uu + 1) * NCOL] = \
                np.broadcast_to(w1, (F, NCOL))
        in_maps.append({"xim": xim, "wbank": wbank, "wx": wxa})
        per_core_meta.append((b, half))
    return in_maps, per_core_meta


def _assemble(results, per_core_meta):
    full = np.empty((B, C, 2, F, TOK), np.float32)
    for core, (b, half) in enumerate(per_core_meta):
        # out[ch, u*32+f, j*16+c] -> full[b, c, ch, f, half*128 + u*32 + j]
        o = np.asarray(results[core]["out"]).reshape(2, U, F, JPU, C)
        o2 = o.transpose(4, 0, 2, 1, 3).reshape(C, 2, F, 128)
        full[b, :, :, :, half * 128:(half + 1) * 128] = o2
    return full


def kernel(x, fs, seq_lens, freqs, n_cycles, target_time_tokens):
    assert int(target_time_tokens) == TOK
    nc = _get_nc()
    in_maps, meta = _host_prep(x, fs, seq_lens, freqs, n_cycles)
    res = run_bass_kernel_spmd(nc, in_maps, list(range(8)))
    return _assemble(res.results, meta)
